# revision 13
# baseline (speedup 1.0000x reference)
"""Trainium2 Bass kernel for nn_Decoder (Bahdanau attention + 2-layer GRU step + vocab projection).

Sharding across 8 NeuronCores:
  - Attention: data-parallel over batch (8 rows per core).
  - GRU (both layers): tensor-parallel over hidden units (64 units per core),
    AllGather of hidden state between stages.
  - fc_out: tensor-parallel column-split over vocab (4000 logits per core).
Host side only reshapes/transposes/slices inputs into matmul-friendly layouts;
all FLOPs (attention, GRU, projection) run on device.
"""

import sys

for _p in ("/opt/trn_rl_repo", "/root/.axon_site/_ro/trn_rl_repo"):
    if _p not in sys.path:
        sys.path.append(_p)

import numpy as np

import concourse.bass as bass
import concourse.mybir as mybir
import concourse.tile as tile
from concourse import bacc
from concourse.bass import ds, ts
from concourse.bass_utils import run_bass_kernel_spmd
from concourse.masks import make_identity

V, E, H, ENC, S, B = 32000, 512, 512, 1024, 128, 64
NCORES = 8
BL = B // NCORES        # 8 batch rows per core
VS = V // NCORES        # 4000 vocab per core
HS = H // NCORES        # 64 hidden units per core
VT = 500                # vocab tile within a core (8 tiles of 500)
F32 = mybir.dt.float32

# compute dtype for matmul operands (weights & stationary activations)
import ml_dtypes
DT_NP = np.float32
DT = F32


def _build_program(dt, n_fcw_bufs):
    nc = bacc.Bacc("TRN2", target_bir_lowering=False, debug=False, num_devices=NCORES)
    f32 = F32

    # ---- I/O ----
    def inp(name, shape, dtype=dt):
        return nc.dram_tensor(name, list(shape), dtype, kind="ExternalInput")

    encT = inp("encT", [8, 128, BL * S])          # [e-chunk, e-part, b*s]
    encN = inp("encN", [128, BL, ENC])            # [s, b, e]
    We = inp("We", [8, 128, H])
    Wd = inp("Wd", [4, 128, H])
    decT = inp("decT", [4, 128, BL])
    biasED = inp("biasED", [1, H])
    attv = inp("attv", [4, 128, 1])
    negm = inp("negm", [1, BL * S], f32)
    embT = inp("embT", [4, 128, B])
    h0T = inp("h0T", [4, 128, B])
    h1T = inp("h1T", [4, 128, B])
    h0prev = inp("h0prev", [B, HS], f32)
    h1prev = inp("h1prev", [B, HS], f32)
    wih0 = inp("wih0", [12, 128, 3 * HS])
    whh0 = inp("whh0", [4, 128, 3 * HS])
    wih1 = inp("wih1", [4, 128, 3 * HS])
    whh1 = inp("whh1", [4, 128, 3 * HS])
    bih0 = inp("bih0", [1, 3 * HS])
    bhh0 = inp("bhh0", [1, 3 * HS])
    bih1 = inp("bih1", [1, 3 * HS])
    bhh1 = inp("bhh1", [1, 3 * HS])
    fcwT = inp("fcwT", [16, 128, VS])
    fcb = inp("fcb", [1, VS])

    logits_o = nc.dram_tensor("logits", [B, VS], f32, kind="ExternalOutput")
    h0_o = nc.dram_tensor("h0o", [B, HS], f32, kind="ExternalOutput")
    h1_o = nc.dram_tensor("h1o", [B, HS], f32, kind="ExternalOutput")
    attn_o = nc.dram_tensor("attno", [BL, S], f32, kind="ExternalOutput")

    RG = [list(range(NCORES))]

    with tile.TileContext(nc) as tc:
        with (
            tc.tile_pool(name="const", bufs=1) as const,
            tc.tile_pool(name="big", bufs=1) as big,
            tc.tile_pool(name="work", bufs=1) as work,
            tc.tile_pool(name="drain", bufs=2) as drainp,
            tc.tile_pool(name="encn", bufs=4) as encn_pool,
            tc.tile_pool(name="fcw", bufs=n_fcw_bufs) as fcw_pool,
            tc.tile_pool(name="psum", bufs=8, space="PSUM") as psum,
            tc.tile_pool(name="dram", bufs=1, space="DRAM") as dram,
        ):
            # ---- constants ----
            ident = const.tile([128, 128], f32)
            make_identity(nc, ident)
            ones = const.tile([1, 128], dt)
            nc.vector.memset(ones, 1.0)

            # ---- load attention operands ----
            encT_sb = big.tile([128, 8, BL * S], dt)
            nc.sync.dma_start(encT_sb, encT[:].rearrange("c p n -> p c n"))
            We_sb = big.tile([128, 8, H], dt)
            nc.sync.dma_start(We_sb, We[:].rearrange("c p n -> p c n"))
            Wd_sb = big.tile([128, 4, H], dt)
            nc.sync.dma_start(Wd_sb, Wd[:].rearrange("c p n -> p c n"))
            decT_sb = big.tile([128, 4, BL], dt)
            nc.sync.dma_start(decT_sb, decT[:].rearrange("c p n -> p c n"))
            biasED_sb = big.tile([1, H], dt)
            nc.sync.dma_start(biasED_sb, biasED[:])
            v_sb = big.tile([128, 4, 1], dt)
            nc.sync.dma_start(v_sb, attv[:].rearrange("c p n -> p c n"))
            negm_sb = big.tile([1, BL * S], f32)
            nc.sync.dma_start(negm_sb, negm[:])

            # ---- dbT[h, b] = (dec @ Wd + be + bd).T ----
            db_ps = psum.tile([128, 512], f32, tag="pb")
            dbv = db_ps[:, 0:4 * BL].rearrange("p (c b) -> p c b", b=BL)
            for i in range(4):
                for j in range(4):
                    nc.tensor.matmul(
                        dbv[:, i, :], lhsT=Wd_sb[:, j, ds(128 * i, 128)],
                        rhs=decT_sb[:, j, :], start=(j == 0), stop=False)
                nc.tensor.matmul(
                    dbv[:, i, :], lhsT=biasED_sb[0:1, ds(128 * i, 128)],
                    rhs=ones[0:1, 0:BL], start=False, stop=True)
            dbT_sb = big.tile([128, 4, BL], f32)
            nc.vector.tensor_copy(dbT_sb, dbv)

            # ---- energy_T = tanh(We.T @ enc + dbT) : [h, b*s] ----
            tanhE_sb = big.tile([128, 4, BL * S], dt)
            for c in range(2):
                for i in range(4):
                    e_ps = psum.tile([128, 512], f32, tag="pb", name="e_ps")
                    for j in range(8):
                        nc.tensor.matmul(
                            e_ps, lhsT=We_sb[:, j, ds(128 * i, 128)],
                            rhs=encT_sb[:, j, ds(512 * c, 512)],
                            start=(j == 0), stop=(j == 7))
                    ev = e_ps[:].rearrange("p (b s) -> p b s", s=S)
                    nc.vector.tensor_tensor(
                        ev, ev,
                        dbT_sb[:, i, ds(4 * c, 4), None].to_broadcast([128, 4, S]),
                        mybir.AluOpType.add)
                    nc.scalar.activation(
                        tanhE_sb[:, i, ds(512 * c, 512)], e_ps,
                        mybir.ActivationFunctionType.Tanh)

            # ---- scores = v.T @ tanh_energy : [1, b*s]; mask; softmax ----
            scores_sb = work.tile([1, BL * S], f32)
            for c in range(2):
                sc_ps = psum.tile([128, 512], f32, tag="pb", name="sc_ps")
                for i in range(4):
                    nc.tensor.matmul(
                        sc_ps[0:1, :], lhsT=v_sb[:, i, :],
                        rhs=tanhE_sb[:, i, ds(512 * c, 512)],
                        start=(i == 0), stop=(i == 3))
                nc.scalar.copy(scores_sb[0:1, ds(512 * c, 512)], sc_ps[0:1, :])
            nc.vector.tensor_add(scores_sb, scores_sb, negm_sb)

            # reshape [1, b*s] -> [b, s] via sbuf-to-sbuf DMA
            scores_bs = work.tile([BL, S], f32)
            nc.sync.dma_start(scores_bs, scores_sb)

            mx = work.tile([BL, 1], f32)
            nc.vector.reduce_max(mx, scores_bs, axis=mybir.AxisListType.X)
            shifted = work.tile([BL, S], f32)
            nc.vector.tensor_scalar(
                shifted, scores_bs, mx, None, mybir.AluOpType.subtract)
            exps = work.tile([BL, S], f32)
            nc.scalar.activation(exps, shifted, mybir.ActivationFunctionType.Exp)
            sm = work.tile([BL, 1], f32)
            nc.vector.reduce_sum(sm, exps, axis=mybir.AxisListType.X)
            rs = work.tile([BL, 1], f32)
            nc.vector.reciprocal(rs, sm)
            attn_sb = work.tile([BL, S], f32)
            nc.vector.tensor_scalar_mul(attn_sb, exps, rs)
            nc.sync.dma_start(attn_o[:], attn_sb)

            # attn.T [s, b] for context matmul
            at_ps = psum.tile([128, 512], f32, tag="pb", name="at_ps")
            nc.tensor.transpose(at_ps[:, 0:BL], attn_sb, ident[0:BL, 0:BL])
            attnT_sb = work.tile([128, BL], dt)
            nc.vector.tensor_copy(attnT_sb, at_ps[:, 0:BL])

            # ---- context[b, e] = attn[b, :] @ enc[b] : via M=8 matmul, take row b ----
            # ---- AllGather context -> [B, ENC]; transpose to ctxT [e, b] ----
            ctx_in = dram.tile([BL, ENC], f32)
            ctx_out = dram.tile([B, ENC], f32, addr_space="Shared")
            for b in range(BL):
                encb = encn_pool.tile([128, ENC], dt, tag="encn")
                nc.sync.dma_start(encb, encN[:, b, :])
                for c2 in range(2):
                    c_ps = psum.tile([128, 512], f32, tag="pb", name="c_ps")
                    nc.tensor.matmul(
                        c_ps[0:BL, :], lhsT=attnT_sb,
                        rhs=encb[:, ds(512 * c2, 512)], start=True, stop=True)
                    cstage = work.tile([BL, 512], f32, name="cstage", bufs=4)
                    nc.vector.tensor_copy(cstage, c_ps[0:BL, :])
                    nc.sync.dma_start(ctx_in[b:b + 1, ds(512 * c2, 512)],
                                      cstage[b:b + 1, :])
            nc.gpsimd.collective_compute(
                "AllGather", mybir.AluOpType.bypass, replica_groups=RG,
                ins=[ctx_in[:].opt()], outs=[ctx_out[:].opt()])
            ctx_full = work.tile([B, ENC], f32)
            nc.gpsimd.dma_start(ctx_full, ctx_out[:])
            ctxT_sb = big.tile([128, 8, B], dt)
            for t in range(8):
                t_ps = psum.tile([128, 512], f32, tag="pb", name="t_ps")
                nc.tensor.transpose(
                    t_ps[:, 0:B], ctx_full[:, ds(128 * t, 128)], ident[0:B, 0:B])
                nc.vector.tensor_copy(ctxT_sb[:, t, :], t_ps[:, 0:B])

            # ---- GRU weights ----
            wih0_sb = big.tile([128, 12, 3 * HS], dt)
            nc.sync.dma_start(wih0_sb, wih0[:].rearrange("c p n -> p c n"))
            whh0_sb = big.tile([128, 4, 3 * HS], dt)
            nc.sync.dma_start(whh0_sb, whh0[:].rearrange("c p n -> p c n"))
            wih1_sb = big.tile([128, 4, 3 * HS], dt)
            nc.sync.dma_start(wih1_sb, wih1[:].rearrange("c p n -> p c n"))
            whh1_sb = big.tile([128, 4, 3 * HS], dt)
            nc.sync.dma_start(whh1_sb, whh1[:].rearrange("c p n -> p c n"))
            embT_sb = big.tile([128, 4, B], dt)
            nc.sync.dma_start(embT_sb, embT[:].rearrange("c p n -> p c n"))
            h0T_sb = big.tile([128, 4, B], dt)
            nc.sync.dma_start(h0T_sb, h0T[:].rearrange("c p n -> p c n"))
            h1T_sb = big.tile([128, 4, B], dt)
            nc.sync.dma_start(h1T_sb, h1T[:].rearrange("c p n -> p c n"))
            bih0_sb = big.tile([1, 3 * HS], dt)
            nc.sync.dma_start(bih0_sb, bih0[:])
            bhh0_sb = big.tile([1, 3 * HS], dt)
            nc.sync.dma_start(bhh0_sb, bhh0[:])
            bih1_sb = big.tile([1, 3 * HS], dt)
            nc.sync.dma_start(bih1_sb, bih1[:])
            bhh1_sb = big.tile([1, 3 * HS], dt)
            nc.sync.dma_start(bhh1_sb, bhh1[:])
            h0prev_sb = work.tile([B, HS], f32, name="h0prev_sb")
            nc.sync.dma_start(h0prev_sb, h0prev[:])
            h1prev_sb = work.tile([B, HS], f32, name="h1prev_sb")
            nc.sync.dma_start(h1prev_sb, h1prev[:])

            def gru_layer(x_chunks, wih_sb, whh_sb, hT_chunks, bih_sb, bhh_sb,
                          hprev_sb, lname):
                gi = psum.tile([128, 512], f32, tag="pb", name=f"gi_{lname}")
                n_x = len(x_chunks)
                for j, xc in enumerate(x_chunks):
                    nc.tensor.matmul(gi[0:B, 0:3 * HS], lhsT=xc, rhs=wih_sb[:, j, :],
                                     start=(j == 0), stop=False)
                nc.tensor.matmul(gi[0:B, 0:3 * HS], lhsT=ones[0:1, 0:B],
                                 rhs=bih_sb[0:1, :], start=False, stop=True)
                gh = psum.tile([128, 512], f32, tag="pb", name=f"gh_{lname}")
                for j, hc in enumerate(hT_chunks):
                    nc.tensor.matmul(gh[0:B, 0:3 * HS], lhsT=hc, rhs=whh_sb[:, j, :],
                                     start=(j == 0), stop=False)
                nc.tensor.matmul(gh[0:B, 0:3 * HS], lhsT=ones[0:1, 0:B],
                                 rhs=bhh_sb[0:1, :], start=False, stop=True)

                Sg = mybir.ActivationFunctionType.Sigmoid
                gh_sb = work.tile([B, 3 * HS], f32, name=f"gh_sb_{lname}")
                nc.scalar.copy(gh_sb, gh[0:B, 0:3 * HS])
                pre_r = work.tile([B, HS], f32, name=f"pre_r_{lname}")
                nc.vector.tensor_add(pre_r, gi[0:B, 0:HS], gh_sb[:, 0:HS])
                r = work.tile([B, HS], f32, name=f"r_{lname}")
                nc.scalar.activation(r, pre_r, Sg)
                pre_z = work.tile([B, HS], f32, name=f"pre_z_{lname}")
                nc.vector.tensor_add(pre_z, gi[0:B, HS:2 * HS], gh_sb[:, HS:2 * HS])
                z = work.tile([B, HS], f32, name=f"z_{lname}")
                nc.scalar.activation(z, pre_z, Sg)
                pre_n = work.tile([B, HS], f32, name=f"pre_n_{lname}")
                nc.vector.tensor_mul(pre_n, r, gh_sb[:, 2 * HS:3 * HS])
                nc.vector.tensor_add(pre_n, pre_n, gi[0:B, 2 * HS:3 * HS])
                n = work.tile([B, HS], f32, name=f"n_{lname}")
                nc.scalar.activation(n, pre_n, mybir.ActivationFunctionType.Tanh)
                hnew = work.tile([B, HS], f32, name=f"hnew_{lname}")
                nc.vector.tensor_sub(hnew, hprev_sb, n)
                nc.vector.tensor_mul(hnew, hnew, z)
                nc.vector.tensor_add(hnew, hnew, n)
                return hnew

            def transpose_ag(h_sb, dram_in, dram_out, out_sb, lname):
                # [B, HS] f32 -> transpose -> cast dt -> AllGather -> [128, 4, B] dt
                tp = psum.tile([128, 512], f32, tag="pb", name=f"tp_{lname}")
                nc.tensor.transpose(tp[0:HS, 0:B], h_sb, ident[0:B, 0:B])
                hT_l = work.tile([HS, B], dt, name=f"hT_{lname}")
                nc.vector.tensor_copy(hT_l, tp[0:HS, 0:B])
                nc.gpsimd.dma_start(dram_in[:], hT_l)
                nc.gpsimd.collective_compute(
                    "AllGather", mybir.AluOpType.bypass, replica_groups=RG,
                    ins=[dram_in[:].opt()], outs=[dram_out[:].opt()])
                nc.gpsimd.dma_start(out_sb, dram_out[:].rearrange("(c p) b -> p c b", p=128))

            # ---- GRU layer 0 (tensor-parallel over hidden units) ----
            x0 = [embT_sb[:, j, :] for j in range(4)] + [ctxT_sb[:, t, :] for t in range(8)]
            h0n = gru_layer(x0, wih0_sb, whh0_sb, [h0T_sb[:, j, :] for j in range(4)],
                            bih0_sb, bhh0_sb, h0prev_sb, "l0")
            nc.sync.dma_start(h0_o[:], h0n)
            h0T_in = dram.tile([HS, B], dt)
            h0T_out = dram.tile([H, B], dt, addr_space="Shared")
            h0nT_sb = big.tile([128, 4, B], dt)
            transpose_ag(h0n, h0T_in, h0T_out, h0nT_sb, "l0")

            # ---- GRU layer 1 ----
            x1 = [h0nT_sb[:, j, :] for j in range(4)]
            h1n = gru_layer(x1, wih1_sb, whh1_sb, [h1T_sb[:, j, :] for j in range(4)],
                            bih1_sb, bhh1_sb, h1prev_sb, "l1")
            nc.sync.dma_start(h1_o[:], h1n)
            h1T_in = dram.tile([HS, B], dt)
            h1T_out = dram.tile([H, B], dt, addr_space="Shared")
            h1nT_sb = big.tile([128, 4, B], dt)
            transpose_ag(h1n, h1T_in, h1T_out, h1nT_sb, "l1")

            # ---- fc_out: logits[b, v] = out_in @ fc_w_slice.T + fc_b ----
            # out_in = [h1 | context | embedded]; chunk order matches host fcwT layout
            fcb_sb = big.tile([1, VS], dt)
            nc.sync.dma_start(fcb_sb, fcb[:])
            chunks = ([h1nT_sb[:, j, :] for j in range(4)]
                      + [ctxT_sb[:, t, :] for t in range(8)]
                      + [embT_sb[:, j, :] for j in range(4)])
            fc_ps = [psum.tile([128, 512], f32, tag="pb", name=f"fc_ps{v}")
                     for v in range(8)]
            for kc in range(16):
                w_t = fcw_pool.tile([128, VS], dt, tag="fcw")
                nc.sync.dma_start(w_t, fcwT[kc])
                for v in range(8):
                    nc.tensor.matmul(
                        fc_ps[v][0:B, 0:VT], lhsT=chunks[kc],
                        rhs=w_t[:, ds(VT * v, VT)], start=(kc == 0), stop=False)
            for v in range(8):
                nc.tensor.matmul(
                    fc_ps[v][0:B, 0:VT], lhsT=ones[0:1, 0:B],
                    rhs=fcb_sb[0:1, ds(VT * v, VT)], start=False, stop=True)
                lg = drainp.tile([B, VT], f32, tag="lg")
                nc.vector.tensor_copy(lg, fc_ps[v][0:B, 0:VT])
                nc.sync.dma_start(logits_o[:, ds(VT * v, VT)], lg)

    nc.finalize()
    return nc


_PROGRAM_CACHE = {}


def _get_program():
    n_fcw_bufs = 2 if DT == F32 else 4
    key = (str(DT), n_fcw_bufs)
    if key not in _PROGRAM_CACHE:
        _PROGRAM_CACHE[key] = _build_program(DT, n_fcw_bufs)
    return _PROGRAM_CACHE[key]


def _prep_core_inputs(i, input_ids, hidden, enc, mask, emb, att_We, att_be,
                      att_Wd, att_bd, att_v, w_ih0, w_hh0, b_ih0, b_hh0,
                      w_ih1, w_hh1, b_ih1, b_hh1, fc_w, fc_b, embedded):
    f4 = np.float32
    bsl = slice(BL * i, BL * (i + 1))
    vsl = slice(VS * i, VS * (i + 1))
    hsl = np.arange(HS * i, HS * (i + 1))
    rows = np.concatenate([hsl, H + hsl, 2 * H + hsl])

    enc_sl = enc[bsl]                                    # [8, S, ENC]
    c = np.ascontiguousarray

    def d(x):
        return c(x.astype(DT_NP))

    m = {}
    m["encT"] = d(enc_sl.transpose(2, 0, 1).reshape(8, 128, BL * S))
    m["encN"] = d(enc_sl.transpose(1, 0, 2))
    m["We"] = d(att_We.reshape(8, 128, H))
    m["Wd"] = d(att_Wd.reshape(4, 128, H))
    m["decT"] = d(hidden[1][bsl].T.reshape(4, 128, BL))
    m["biasED"] = d((att_be + att_bd).reshape(1, H))
    m["attv"] = d(att_v.reshape(4, 128, 1))
    m["negm"] = c(np.where(mask[bsl] > 0, 0.0, -1e9).astype(f4).reshape(1, BL * S))
    m["embT"] = d(embedded.T.reshape(4, 128, B))
    m["h0T"] = d(hidden[0].T.reshape(4, 128, B))
    m["h1T"] = d(hidden[1].T.reshape(4, 128, B))
    m["h0prev"] = c(hidden[0][:, hsl].astype(f4))
    m["h1prev"] = c(hidden[1][:, hsl].astype(f4))
    m["wih0"] = d(w_ih0[rows].T.reshape(12, 128, 3 * HS))
    m["whh0"] = d(w_hh0[rows].T.reshape(4, 128, 3 * HS))
    m["wih1"] = d(w_ih1[rows].T.reshape(4, 128, 3 * HS))
    m["whh1"] = d(w_hh1[rows].T.reshape(4, 128, 3 * HS))
    m["bih0"] = d(b_ih0[rows].reshape(1, 3 * HS))
    m["bhh0"] = d(b_hh0[rows].reshape(1, 3 * HS))
    m["bih1"] = d(b_ih1[rows].reshape(1, 3 * HS))
    m["bhh1"] = d(b_hh1[rows].reshape(1, 3 * HS))
    # fc_w columns: [h1 | context | embedded] == natural out_in order
    m["fcwT"] = d(fc_w[vsl].T.reshape(16, 128, VS))
    m["fcb"] = d(fc_b[vsl].reshape(1, VS))
    return m


def kernel(input_ids, hidden, encoder_outputs, mask, emb, att_We, att_be,
           att_Wd, att_bd, att_v, w_ih0, w_hh0, b_ih0, b_hh0,
           w_ih1, w_hh1, b_ih1, b_hh1, fc_w, fc_b, _trace=False):
    f4 = np.float32
    input_ids = np.asarray(input_ids)
    hidden = np.asarray(hidden, dtype=f4)
    enc = np.asarray(encoder_outputs, dtype=f4)
    mask = np.asarray(mask)
    emb = np.asarray(emb, dtype=f4)
    att_We = np.asarray(att_We, dtype=f4)
    att_be = np.asarray(att_be, dtype=f4)
    att_Wd = np.asarray(att_Wd, dtype=f4)
    att_bd = np.asarray(att_bd, dtype=f4)
    att_v = np.asarray(att_v, dtype=f4)
    w_ih0 = np.asarray(w_ih0, dtype=f4)
    w_hh0 = np.asarray(w_hh0, dtype=f4)
    b_ih0 = np.asarray(b_ih0, dtype=f4)
    b_hh0 = np.asarray(b_hh0, dtype=f4)
    w_ih1 = np.asarray(w_ih1, dtype=f4)
    w_hh1 = np.asarray(w_hh1, dtype=f4)
    b_ih1 = np.asarray(b_ih1, dtype=f4)
    b_hh1 = np.asarray(b_hh1, dtype=f4)
    fc_w = np.asarray(fc_w, dtype=f4)
    fc_b = np.asarray(fc_b, dtype=f4)

    embedded = emb[input_ids.astype(np.int64)]           # [B, E] host gather

    nc = _get_program()
    in_maps = [
        _prep_core_inputs(i, input_ids, hidden, enc, mask, emb, att_We, att_be,
                          att_Wd, att_bd, att_v, w_ih0, w_hh0, b_ih0, b_hh0,
                          w_ih1, w_hh1, b_ih1, b_hh1, fc_w, fc_b, embedded)
        for i in range(NCORES)
    ]
    res = run_bass_kernel_spmd(nc, in_maps, core_ids=list(range(NCORES)),
                               trace=_trace)
    outs = res.results

    logits = np.concatenate([outs[i]["logits"] for i in range(NCORES)], axis=1)
    h0 = np.concatenate([outs[i]["h0o"] for i in range(NCORES)], axis=1)
    h1 = np.concatenate([outs[i]["h1o"] for i in range(NCORES)], axis=1)
    new_hidden = np.stack([h0, h1], axis=0)
    attn = np.concatenate([outs[i]["attno"] for i in range(NCORES)], axis=0)

    kernel._last_exec_time_ns = res.exec_time_ns
    return logits.astype(f4), new_hidden.astype(f4), attn.astype(f4)


# revision 15
# speedup vs baseline: 1.9606x; 1.9606x over previous
"""Trainium2 Bass kernel for nn_Decoder (Bahdanau attention + 2-layer GRU step + vocab projection).

Sharding across 8 NeuronCores:
  - Attention: data-parallel over batch (8 rows per core).
  - GRU (both layers): tensor-parallel over hidden units (64 units per core),
    AllGather of hidden state between stages.
  - fc_out: tensor-parallel column-split over vocab (4000 logits per core).
Host side only reshapes/transposes/slices inputs into matmul-friendly layouts;
all FLOPs (attention, GRU, projection) run on device.
"""

import sys

for _p in ("/opt/trn_rl_repo", "/root/.axon_site/_ro/trn_rl_repo"):
    if _p not in sys.path:
        sys.path.append(_p)

import numpy as np

import concourse.bass as bass
import concourse.mybir as mybir
import concourse.tile as tile
from concourse import bacc
from concourse.bass import ds, ts
from concourse.bass_utils import run_bass_kernel_spmd
from concourse.masks import make_identity

V, E, H, ENC, S, B = 32000, 512, 512, 1024, 128, 64
NCORES = 8
BL = B // NCORES        # 8 batch rows per core
VS = V // NCORES        # 4000 vocab per core
HS = H // NCORES        # 64 hidden units per core
VT = 500                # vocab tile within a core (8 tiles of 500)
F32 = mybir.dt.float32

# compute dtype for matmul operands (weights & stationary activations)
import ml_dtypes
BF16 = mybir.dt.bfloat16
DT_NP = ml_dtypes.bfloat16
DT = BF16


def _build_program(dt, n_fcw_bufs):
    nc = bacc.Bacc("TRN2", target_bir_lowering=False, debug=False, num_devices=NCORES)
    f32 = F32

    # ---- I/O ----
    def inp(name, shape, dtype=dt):
        return nc.dram_tensor(name, list(shape), dtype, kind="ExternalInput")

    encT = inp("encT", [8, 128, BL * S])          # [e-chunk, e-part, b*s]
    encN = inp("encN", [128, BL, ENC])            # [s, b, e]
    We = inp("We", [8, 128, H])
    Wd = inp("Wd", [4, 128, H])
    decT = inp("decT", [4, 128, BL])
    biasED = inp("biasED", [1, H])
    attv = inp("attv", [4, 128, 1])
    negm = inp("negm", [1, BL * S], f32)
    embT = inp("embT", [4, 128, B])
    h0T = inp("h0T", [4, 128, B])
    h1T = inp("h1T", [4, 128, B])
    h0prev = inp("h0prev", [B, HS], f32)
    h1prev = inp("h1prev", [B, HS], f32)
    wih0 = inp("wih0", [12, 128, 3 * HS])
    whh0 = inp("whh0", [4, 128, 3 * HS])
    wih1 = inp("wih1", [4, 128, 3 * HS])
    whh1 = inp("whh1", [4, 128, 3 * HS])
    bih0 = inp("bih0", [1, 3 * HS])
    bhh0 = inp("bhh0", [1, 3 * HS])
    bih1 = inp("bih1", [1, 3 * HS])
    bhh1 = inp("bhh1", [1, 3 * HS])
    fcwT = inp("fcwT", [16, 128, VS])
    fcb = inp("fcb", [1, VS])

    logits_o = nc.dram_tensor("logits", [B, VS], f32, kind="ExternalOutput")
    h0_o = nc.dram_tensor("h0o", [B, HS], f32, kind="ExternalOutput")
    h1_o = nc.dram_tensor("h1o", [B, HS], f32, kind="ExternalOutput")
    attn_o = nc.dram_tensor("attno", [BL, S], f32, kind="ExternalOutput")

    RG = [list(range(NCORES))]

    with tile.TileContext(nc) as tc:
        with (
            tc.tile_pool(name="const", bufs=1) as const,
            tc.tile_pool(name="big", bufs=1) as big,
            tc.tile_pool(name="work", bufs=1) as work,
            tc.tile_pool(name="drain", bufs=2) as drainp,
            tc.tile_pool(name="encn", bufs=4) as encn_pool,
            tc.tile_pool(name="fcw", bufs=n_fcw_bufs) as fcw_pool,
            tc.tile_pool(name="psum", bufs=8, space="PSUM") as psum,
            tc.tile_pool(name="dram", bufs=1, space="DRAM") as dram,
        ):
            # ---- constants ----
            ident = const.tile([128, 128], f32)
            make_identity(nc, ident)
            ones = const.tile([1, 128], dt)
            nc.vector.memset(ones, 1.0)

            # ---- load attention operands ----
            encT_sb = big.tile([128, 8, BL * S], dt)
            nc.sync.dma_start(encT_sb, encT[:].rearrange("c p n -> p c n"))
            We_sb = big.tile([128, 8, H], dt)
            nc.sync.dma_start(We_sb, We[:].rearrange("c p n -> p c n"))
            Wd_sb = big.tile([128, 4, H], dt)
            nc.sync.dma_start(Wd_sb, Wd[:].rearrange("c p n -> p c n"))
            decT_sb = big.tile([128, 4, BL], dt)
            nc.sync.dma_start(decT_sb, decT[:].rearrange("c p n -> p c n"))
            biasED_sb = big.tile([1, H], dt)
            nc.sync.dma_start(biasED_sb, biasED[:])
            v_sb = big.tile([128, 4, 1], dt)
            nc.sync.dma_start(v_sb, attv[:].rearrange("c p n -> p c n"))
            negm_sb = big.tile([1, BL * S], f32)
            nc.sync.dma_start(negm_sb, negm[:])

            # ---- dbT[h, b] = (dec @ Wd + be + bd).T ----
            db_ps = psum.tile([128, 512], f32, tag="pb")
            dbv = db_ps[:, 0:4 * BL].rearrange("p (c b) -> p c b", b=BL)
            for i in range(4):
                for j in range(4):
                    nc.tensor.matmul(
                        dbv[:, i, :], lhsT=Wd_sb[:, j, ds(128 * i, 128)],
                        rhs=decT_sb[:, j, :], start=(j == 0), stop=False)
                nc.tensor.matmul(
                    dbv[:, i, :], lhsT=biasED_sb[0:1, ds(128 * i, 128)],
                    rhs=ones[0:1, 0:BL], start=False, stop=True)
            dbT_sb = big.tile([128, 4, BL], f32)
            nc.vector.tensor_copy(dbT_sb, dbv)

            # ---- energy_T = tanh(We.T @ enc + dbT) : [h, b*s] ----
            tanhE_sb = big.tile([128, 4, BL * S], dt)
            for c in range(2):
                for i in range(4):
                    e_ps = psum.tile([128, 512], f32, tag="pb", name="e_ps")
                    for j in range(8):
                        nc.tensor.matmul(
                            e_ps, lhsT=We_sb[:, j, ds(128 * i, 128)],
                            rhs=encT_sb[:, j, ds(512 * c, 512)],
                            start=(j == 0), stop=(j == 7))
                    ev = e_ps[:].rearrange("p (b s) -> p b s", s=S)
                    nc.vector.tensor_tensor(
                        ev, ev,
                        dbT_sb[:, i, ds(4 * c, 4), None].to_broadcast([128, 4, S]),
                        mybir.AluOpType.add)
                    nc.scalar.activation(
                        tanhE_sb[:, i, ds(512 * c, 512)], e_ps,
                        mybir.ActivationFunctionType.Tanh)

            # ---- scores = v.T @ tanh_energy : [1, b*s]; mask; softmax ----
            scores_sb = work.tile([1, BL * S], f32)
            for c in range(2):
                sc_ps = psum.tile([128, 512], f32, tag="pb", name="sc_ps")
                for i in range(4):
                    nc.tensor.matmul(
                        sc_ps[0:1, :], lhsT=v_sb[:, i, :],
                        rhs=tanhE_sb[:, i, ds(512 * c, 512)],
                        start=(i == 0), stop=(i == 3))
                nc.scalar.copy(scores_sb[0:1, ds(512 * c, 512)], sc_ps[0:1, :])
            nc.vector.tensor_add(scores_sb, scores_sb, negm_sb)

            # reshape [1, b*s] -> [b, s] via sbuf-to-sbuf DMA
            scores_bs = work.tile([BL, S], f32)
            nc.gpsimd.dma_start(scores_bs, scores_sb)

            mx = work.tile([BL, 1], f32)
            nc.vector.reduce_max(mx, scores_bs, axis=mybir.AxisListType.X)
            shifted = work.tile([BL, S], f32)
            nc.vector.tensor_scalar(
                shifted, scores_bs, mx, None, mybir.AluOpType.subtract)
            exps = work.tile([BL, S], f32)
            nc.scalar.activation(exps, shifted, mybir.ActivationFunctionType.Exp)
            sm = work.tile([BL, 1], f32)
            nc.vector.reduce_sum(sm, exps, axis=mybir.AxisListType.X)
            rs = work.tile([BL, 1], f32)
            nc.vector.reciprocal(rs, sm)
            attn_sb = work.tile([BL, S], f32)
            nc.vector.tensor_scalar_mul(attn_sb, exps, rs)
            nc.gpsimd.dma_start(attn_o[:], attn_sb)

            # attn.T [s, b] for context matmul
            at_ps = psum.tile([128, 512], f32, tag="pb", name="at_ps")
            nc.tensor.transpose(at_ps[:, 0:BL], attn_sb, ident[0:BL, 0:BL])
            attnT_sb = work.tile([128, BL], dt)
            nc.vector.tensor_copy(attnT_sb, at_ps[:, 0:BL])

            # ---- context[b, e] = attn[b, :] @ enc[b] : via M=8 matmul, take row b ----
            # ---- AllGather context -> [B, ENC]; transpose to ctxT [e, b] ----
            ctx_in = dram.tile([BL, ENC], f32)
            ctx_out = dram.tile([B, ENC], f32, addr_space="Shared")
            for b in range(BL):
                encb = encn_pool.tile([128, ENC], dt, tag="encn")
                nc.sync.dma_start(encb, encN[:, b, :])
                for c2 in range(2):
                    c_ps = psum.tile([128, 512], f32, tag="pb", name="c_ps")
                    nc.tensor.matmul(
                        c_ps[0:BL, :], lhsT=attnT_sb,
                        rhs=encb[:, ds(512 * c2, 512)], start=True, stop=True)
                    cstage = work.tile([BL, 512], f32, name="cstage", bufs=4)
                    nc.vector.tensor_copy(cstage, c_ps[0:BL, :])
                    nc.gpsimd.dma_start(ctx_in[b:b + 1, ds(512 * c2, 512)],
                                          cstage[b:b + 1, :])
            nc.gpsimd.collective_compute(
                "AllGather", mybir.AluOpType.bypass, replica_groups=RG,
                ins=[ctx_in[:].opt()], outs=[ctx_out[:].opt()])
            ctx_full = work.tile([B, ENC], f32)
            nc.gpsimd.dma_start(ctx_full, ctx_out[:])
            ctxT_sb = big.tile([128, 8, B], dt)
            for t in range(8):
                t_ps = psum.tile([128, 512], f32, tag="pb", name="t_ps")
                nc.tensor.transpose(
                    t_ps[:, 0:B], ctx_full[:, ds(128 * t, 128)], ident[0:B, 0:B])
                nc.vector.tensor_copy(ctxT_sb[:, t, :], t_ps[:, 0:B])

            # ---- GRU weights ----
            wih0_sb = big.tile([128, 12, 3 * HS], dt)
            nc.sync.dma_start(wih0_sb, wih0[:].rearrange("c p n -> p c n"))
            whh0_sb = big.tile([128, 4, 3 * HS], dt)
            nc.sync.dma_start(whh0_sb, whh0[:].rearrange("c p n -> p c n"))
            wih1_sb = big.tile([128, 4, 3 * HS], dt)
            nc.sync.dma_start(wih1_sb, wih1[:].rearrange("c p n -> p c n"))
            whh1_sb = big.tile([128, 4, 3 * HS], dt)
            nc.sync.dma_start(whh1_sb, whh1[:].rearrange("c p n -> p c n"))
            embT_sb = big.tile([128, 4, B], dt)
            nc.sync.dma_start(embT_sb, embT[:].rearrange("c p n -> p c n"))
            h0T_sb = big.tile([128, 4, B], dt)
            nc.sync.dma_start(h0T_sb, h0T[:].rearrange("c p n -> p c n"))
            h1T_sb = big.tile([128, 4, B], dt)
            nc.sync.dma_start(h1T_sb, h1T[:].rearrange("c p n -> p c n"))
            bih0_sb = big.tile([1, 3 * HS], dt)
            nc.sync.dma_start(bih0_sb, bih0[:])
            bhh0_sb = big.tile([1, 3 * HS], dt)
            nc.sync.dma_start(bhh0_sb, bhh0[:])
            bih1_sb = big.tile([1, 3 * HS], dt)
            nc.sync.dma_start(bih1_sb, bih1[:])
            bhh1_sb = big.tile([1, 3 * HS], dt)
            nc.sync.dma_start(bhh1_sb, bhh1[:])
            h0prev_sb = work.tile([B, HS], f32, name="h0prev_sb")
            nc.sync.dma_start(h0prev_sb, h0prev[:])
            h1prev_sb = work.tile([B, HS], f32, name="h1prev_sb")
            nc.sync.dma_start(h1prev_sb, h1prev[:])

            def gru_layer(x_chunks, wih_sb, whh_sb, hT_chunks, bih_sb, bhh_sb,
                          hprev_sb, lname):
                gi = psum.tile([128, 512], f32, tag="pb", name=f"gi_{lname}")
                n_x = len(x_chunks)
                for j, xc in enumerate(x_chunks):
                    nc.tensor.matmul(gi[0:B, 0:3 * HS], lhsT=xc, rhs=wih_sb[:, j, :],
                                     start=(j == 0), stop=False)
                nc.tensor.matmul(gi[0:B, 0:3 * HS], lhsT=ones[0:1, 0:B],
                                 rhs=bih_sb[0:1, :], start=False, stop=True)
                gh = psum.tile([128, 512], f32, tag="pb", name=f"gh_{lname}")
                for j, hc in enumerate(hT_chunks):
                    nc.tensor.matmul(gh[0:B, 0:3 * HS], lhsT=hc, rhs=whh_sb[:, j, :],
                                     start=(j == 0), stop=False)
                nc.tensor.matmul(gh[0:B, 0:3 * HS], lhsT=ones[0:1, 0:B],
                                 rhs=bhh_sb[0:1, :], start=False, stop=True)

                Sg = mybir.ActivationFunctionType.Sigmoid
                gh_sb = work.tile([B, 3 * HS], f32, name=f"gh_sb_{lname}")
                nc.scalar.copy(gh_sb, gh[0:B, 0:3 * HS])
                pre_r = work.tile([B, HS], f32, name=f"pre_r_{lname}")
                nc.vector.tensor_add(pre_r, gi[0:B, 0:HS], gh_sb[:, 0:HS])
                r = work.tile([B, HS], f32, name=f"r_{lname}")
                nc.scalar.activation(r, pre_r, Sg)
                pre_z = work.tile([B, HS], f32, name=f"pre_z_{lname}")
                nc.vector.tensor_add(pre_z, gi[0:B, HS:2 * HS], gh_sb[:, HS:2 * HS])
                z = work.tile([B, HS], f32, name=f"z_{lname}")
                nc.scalar.activation(z, pre_z, Sg)
                pre_n = work.tile([B, HS], f32, name=f"pre_n_{lname}")
                nc.vector.tensor_mul(pre_n, r, gh_sb[:, 2 * HS:3 * HS])
                nc.vector.tensor_add(pre_n, pre_n, gi[0:B, 2 * HS:3 * HS])
                n = work.tile([B, HS], f32, name=f"n_{lname}")
                nc.scalar.activation(n, pre_n, mybir.ActivationFunctionType.Tanh)
                hnew = work.tile([B, HS], f32, name=f"hnew_{lname}")
                nc.vector.tensor_sub(hnew, hprev_sb, n)
                nc.vector.tensor_mul(hnew, hnew, z)
                nc.vector.tensor_add(hnew, hnew, n)
                return hnew

            def transpose_ag(h_sb, dram_in, dram_out, out_sb, lname):
                # [B, HS] f32 -> transpose -> cast dt -> AllGather -> [128, 4, B] dt
                tp = psum.tile([128, 512], f32, tag="pb", name=f"tp_{lname}")
                nc.tensor.transpose(tp[0:HS, 0:B], h_sb, ident[0:B, 0:B])
                hT_l = work.tile([HS, B], dt, name=f"hT_{lname}")
                nc.vector.tensor_copy(hT_l, tp[0:HS, 0:B])
                nc.gpsimd.dma_start(dram_in[:], hT_l)
                nc.gpsimd.collective_compute(
                    "AllGather", mybir.AluOpType.bypass, replica_groups=RG,
                    ins=[dram_in[:].opt()], outs=[dram_out[:].opt()])
                nc.gpsimd.dma_start(out_sb, dram_out[:].rearrange("(c p) b -> p c b", p=128))

            # ---- GRU layer 0 (tensor-parallel over hidden units) ----
            x0 = [embT_sb[:, j, :] for j in range(4)] + [ctxT_sb[:, t, :] for t in range(8)]
            h0n = gru_layer(x0, wih0_sb, whh0_sb, [h0T_sb[:, j, :] for j in range(4)],
                            bih0_sb, bhh0_sb, h0prev_sb, "l0")
            nc.gpsimd.dma_start(h0_o[:], h0n)
            h0T_in = dram.tile([HS, B], dt)
            h0T_out = dram.tile([H, B], dt, addr_space="Shared")
            h0nT_sb = big.tile([128, 4, B], dt)
            transpose_ag(h0n, h0T_in, h0T_out, h0nT_sb, "l0")

            # ---- GRU layer 1 ----
            x1 = [h0nT_sb[:, j, :] for j in range(4)]
            h1n = gru_layer(x1, wih1_sb, whh1_sb, [h1T_sb[:, j, :] for j in range(4)],
                            bih1_sb, bhh1_sb, h1prev_sb, "l1")
            nc.gpsimd.dma_start(h1_o[:], h1n)
            h1T_in = dram.tile([HS, B], dt)
            h1T_out = dram.tile([H, B], dt, addr_space="Shared")
            h1nT_sb = big.tile([128, 4, B], dt)
            transpose_ag(h1n, h1T_in, h1T_out, h1nT_sb, "l1")

            # ---- fc_out: logits[b, v] = out_in @ fc_w_slice.T + fc_b ----
            # out_in = [h1 | context | embedded]; chunk order matches host fcwT layout
            fcb_sb = big.tile([1, VS], dt)
            nc.sync.dma_start(fcb_sb, fcb[:])
            chunks = ([h1nT_sb[:, j, :] for j in range(4)]
                      + [ctxT_sb[:, t, :] for t in range(8)]
                      + [embT_sb[:, j, :] for j in range(4)])
            fc_ps = [psum.tile([128, 512], f32, tag="pb", name=f"fc_ps{v}")
                     for v in range(8)]
            for kc in range(16):
                w_t = fcw_pool.tile([128, VS], dt, tag="fcw")
                nc.sync.dma_start(w_t, fcwT[kc])
                for v in range(8):
                    nc.tensor.matmul(
                        fc_ps[v][0:B, 0:VT], lhsT=chunks[kc],
                        rhs=w_t[:, ds(VT * v, VT)], start=(kc == 0), stop=False)
            for v in range(8):
                nc.tensor.matmul(
                    fc_ps[v][0:B, 0:VT], lhsT=ones[0:1, 0:B],
                    rhs=fcb_sb[0:1, ds(VT * v, VT)], start=False, stop=True)
                lg = drainp.tile([B, VT], f32, tag="lg")
                nc.vector.tensor_copy(lg, fc_ps[v][0:B, 0:VT])
                nc.sync.dma_start(logits_o[:, ds(VT * v, VT)], lg)

    nc.finalize()
    return nc


_PROGRAM_CACHE = {}


def _get_program():
    n_fcw_bufs = 2 if DT == F32 else 10
    key = (str(DT), n_fcw_bufs)
    if key not in _PROGRAM_CACHE:
        _PROGRAM_CACHE[key] = _build_program(DT, n_fcw_bufs)
    return _PROGRAM_CACHE[key]


def _prep_core_inputs(i, input_ids, hidden, enc, mask, emb, att_We, att_be,
                      att_Wd, att_bd, att_v, w_ih0, w_hh0, b_ih0, b_hh0,
                      w_ih1, w_hh1, b_ih1, b_hh1, fc_w, fc_b, embedded):
    f4 = np.float32
    bsl = slice(BL * i, BL * (i + 1))
    vsl = slice(VS * i, VS * (i + 1))
    hsl = np.arange(HS * i, HS * (i + 1))
    rows = np.concatenate([hsl, H + hsl, 2 * H + hsl])

    enc_sl = enc[bsl]                                    # [8, S, ENC]
    c = np.ascontiguousarray

    def d(x):
        return c(x.astype(DT_NP))

    m = {}
    m["encT"] = d(enc_sl.transpose(2, 0, 1).reshape(8, 128, BL * S))
    m["encN"] = d(enc_sl.transpose(1, 0, 2))
    m["We"] = d(att_We.reshape(8, 128, H))
    m["Wd"] = d(att_Wd.reshape(4, 128, H))
    m["decT"] = d(hidden[1][bsl].T.reshape(4, 128, BL))
    m["biasED"] = d((att_be + att_bd).reshape(1, H))
    m["attv"] = d(att_v.reshape(4, 128, 1))
    m["negm"] = c(np.where(mask[bsl] > 0, 0.0, -1e9).astype(f4).reshape(1, BL * S))
    m["embT"] = d(embedded.T.reshape(4, 128, B))
    m["h0T"] = d(hidden[0].T.reshape(4, 128, B))
    m["h1T"] = d(hidden[1].T.reshape(4, 128, B))
    m["h0prev"] = c(hidden[0][:, hsl].astype(f4))
    m["h1prev"] = c(hidden[1][:, hsl].astype(f4))
    m["wih0"] = d(w_ih0[rows].T.reshape(12, 128, 3 * HS))
    m["whh0"] = d(w_hh0[rows].T.reshape(4, 128, 3 * HS))
    m["wih1"] = d(w_ih1[rows].T.reshape(4, 128, 3 * HS))
    m["whh1"] = d(w_hh1[rows].T.reshape(4, 128, 3 * HS))
    m["bih0"] = d(b_ih0[rows].reshape(1, 3 * HS))
    m["bhh0"] = d(b_hh0[rows].reshape(1, 3 * HS))
    m["bih1"] = d(b_ih1[rows].reshape(1, 3 * HS))
    m["bhh1"] = d(b_hh1[rows].reshape(1, 3 * HS))
    # fc_w columns: [h1 | context | embedded] == natural out_in order
    m["fcwT"] = d(fc_w[vsl].T.reshape(16, 128, VS))
    m["fcb"] = d(fc_b[vsl].reshape(1, VS))
    return m


def kernel(input_ids, hidden, encoder_outputs, mask, emb, att_We, att_be,
           att_Wd, att_bd, att_v, w_ih0, w_hh0, b_ih0, b_hh0,
           w_ih1, w_hh1, b_ih1, b_hh1, fc_w, fc_b, _trace=False):
    f4 = np.float32
    input_ids = np.asarray(input_ids)
    hidden = np.asarray(hidden, dtype=f4)
    enc = np.asarray(encoder_outputs, dtype=f4)
    mask = np.asarray(mask)
    emb = np.asarray(emb, dtype=f4)
    att_We = np.asarray(att_We, dtype=f4)
    att_be = np.asarray(att_be, dtype=f4)
    att_Wd = np.asarray(att_Wd, dtype=f4)
    att_bd = np.asarray(att_bd, dtype=f4)
    att_v = np.asarray(att_v, dtype=f4)
    w_ih0 = np.asarray(w_ih0, dtype=f4)
    w_hh0 = np.asarray(w_hh0, dtype=f4)
    b_ih0 = np.asarray(b_ih0, dtype=f4)
    b_hh0 = np.asarray(b_hh0, dtype=f4)
    w_ih1 = np.asarray(w_ih1, dtype=f4)
    w_hh1 = np.asarray(w_hh1, dtype=f4)
    b_ih1 = np.asarray(b_ih1, dtype=f4)
    b_hh1 = np.asarray(b_hh1, dtype=f4)
    fc_w = np.asarray(fc_w, dtype=f4)
    fc_b = np.asarray(fc_b, dtype=f4)

    embedded = emb[input_ids.astype(np.int64)]           # [B, E] host gather

    nc = _get_program()
    in_maps = [
        _prep_core_inputs(i, input_ids, hidden, enc, mask, emb, att_We, att_be,
                          att_Wd, att_bd, att_v, w_ih0, w_hh0, b_ih0, b_hh0,
                          w_ih1, w_hh1, b_ih1, b_hh1, fc_w, fc_b, embedded)
        for i in range(NCORES)
    ]
    res = run_bass_kernel_spmd(nc, in_maps, core_ids=list(range(NCORES)),
                               trace=_trace)
    outs = res.results

    logits = np.concatenate([outs[i]["logits"] for i in range(NCORES)], axis=1)
    h0 = np.concatenate([outs[i]["h0o"] for i in range(NCORES)], axis=1)
    h1 = np.concatenate([outs[i]["h1o"] for i in range(NCORES)], axis=1)
    new_hidden = np.stack([h0, h1], axis=0)
    attn = np.concatenate([outs[i]["attno"] for i in range(NCORES)], axis=0)

    kernel._last_exec_time_ns = res.exec_time_ns
    kernel._last_res = res
    return logits.astype(f4), new_hidden.astype(f4), attn.astype(f4)


# revision 16
# speedup vs baseline: 2.3356x; 1.1912x over previous
"""Trainium2 Bass kernel for nn_Decoder (Bahdanau attention + 2-layer GRU step + vocab projection).

Sharding across 8 NeuronCores:
  - Attention: data-parallel over batch (8 rows per core).
  - GRU (both layers): tensor-parallel over hidden units (64 units per core),
    AllGather of hidden state between stages.
  - fc_out: tensor-parallel column-split over vocab (4000 logits per core).
Host side only reshapes/transposes/slices inputs into matmul-friendly layouts;
all FLOPs (attention, GRU, projection) run on device.
"""

import sys

for _p in ("/opt/trn_rl_repo", "/root/.axon_site/_ro/trn_rl_repo"):
    if _p not in sys.path:
        sys.path.append(_p)

import numpy as np

import concourse.bass as bass
import concourse.mybir as mybir
import concourse.tile as tile
from concourse import bacc
from concourse.bass import ds, ts
from concourse.bass_utils import run_bass_kernel_spmd
from concourse.masks import make_identity

V, E, H, ENC, S, B = 32000, 512, 512, 1024, 128, 64
NCORES = 8
BL = B // NCORES        # 8 batch rows per core
VS = V // NCORES        # 4000 vocab per core
HS = H // NCORES        # 64 hidden units per core
VT = 500                # vocab tile within a core (8 tiles of 500)
F32 = mybir.dt.float32

# compute dtype for matmul operands (weights & stationary activations)
import ml_dtypes
BF16 = mybir.dt.bfloat16
DT_NP = ml_dtypes.bfloat16
DT = BF16


def _build_program(dt, n_fcw_bufs):
    nc = bacc.Bacc("TRN2", target_bir_lowering=False, debug=False, num_devices=NCORES)
    f32 = F32

    # ---- I/O ----
    def inp(name, shape, dtype=dt):
        return nc.dram_tensor(name, list(shape), dtype, kind="ExternalInput")

    encT = inp("encT", [8, 128, BL * S])          # [e-chunk, e-part, b*s]
    encN = inp("encN", [128, BL, ENC])            # [s, b, e]
    We = inp("We", [8, 128, H])
    # attp packs Wd | decT | attv along the free axis: [4, 128, 512 + BL + 1]
    attp = inp("attp", [4, 128, H + BL + 1])
    biasED = inp("biasED", [1, H])
    negm = inp("negm", [1, BL * S], f32)
    embT = inp("embT", [4, 128, B])
    h0T = inp("h0T", [4, 128, B])
    h1T = inp("h1T", [4, 128, B])
    h0prev = inp("h0prev", [B, HS], f32)
    h1prev = inp("h1prev", [B, HS], f32)
    wih0 = inp("wih0", [12, 128, 3 * HS])
    whh0 = inp("whh0", [4, 128, 3 * HS])
    wih1 = inp("wih1", [4, 128, 3 * HS])
    whh1 = inp("whh1", [4, 128, 3 * HS])
    # gbias packs bih0 | bhh0 | bih1 | bhh1: [1, 4*192]
    gbias = inp("gbias", [1, 4 * 3 * HS])
    fcwT = inp("fcwT", [16, 128, VS])
    fcb = inp("fcb", [1, VS])

    logits_o = nc.dram_tensor("logits", [B, VS], f32, kind="ExternalOutput")
    h0_o = nc.dram_tensor("h0o", [B, HS], f32, kind="ExternalOutput")
    h1_o = nc.dram_tensor("h1o", [B, HS], f32, kind="ExternalOutput")
    attn_o = nc.dram_tensor("attno", [BL, S], f32, kind="ExternalOutput")

    RG = [list(range(NCORES))]

    def AG(i, o):
        nc.gpsimd.collective_compute(
            "AllGather", mybir.AluOpType.bypass, replica_groups=RG,
            ins=[i[:].opt()], outs=[o[:].opt()])

    with tile.TileContext(nc) as tc:
        with (
            tc.tile_pool(name="const", bufs=1) as const,
            tc.tile_pool(name="big", bufs=1) as big,
            tc.tile_pool(name="work", bufs=1) as work,
            tc.tile_pool(name="drain", bufs=2) as drainp,
            tc.tile_pool(name="encn", bufs=4) as encn_pool,
            tc.tile_pool(name="fcw", bufs=n_fcw_bufs) as fcw_pool,
            tc.tile_pool(name="psum", bufs=8, space="PSUM") as psum,
            tc.tile_pool(name="dram", bufs=1, space="DRAM") as dram,
        ):
            Sg = mybir.ActivationFunctionType.Sigmoid
            Th = mybir.ActivationFunctionType.Tanh

            # ---- dummy collective to absorb core-launch skew (overlaps attention)
            skew_in = dram.tile([1, 16], f32)
            skew_out = dram.tile([NCORES, 16], f32, addr_space="Shared")
            AG(skew_in, skew_out)

            # ---- constants ----
            ident = const.tile([128, 128], f32)
            make_identity(nc, ident)
            ones = const.tile([1, 128], dt)
            nc.vector.memset(ones, 1.0)

            # ---- load attention + GRU operands ----
            encT_sb = big.tile([128, 8, BL * S], dt)
            nc.sync.dma_start(encT_sb, encT[:].rearrange("c p n -> p c n"))
            We_sb = big.tile([128, 8, H], dt)
            nc.sync.dma_start(We_sb, We[:].rearrange("c p n -> p c n"))
            attp_sb = big.tile([128, 4, H + BL + 1], dt)
            nc.sync.dma_start(attp_sb, attp[:].rearrange("c p n -> p c n"))
            Wd_sb = attp_sb[:, :, 0:H]
            decT_sb = attp_sb[:, :, H:H + BL]
            v_sb = attp_sb[:, :, H + BL:H + BL + 1]
            biasED_sb = big.tile([1, H], dt)
            nc.sync.dma_start(biasED_sb, biasED[:])
            negm_sb = big.tile([1, BL * S], f32)
            nc.sync.dma_start(negm_sb, negm[:])

            embT_sb = big.tile([128, 4, B], dt)
            nc.sync.dma_start(embT_sb, embT[:].rearrange("c p n -> p c n"))
            h0T_sb = big.tile([128, 4, B], dt)
            nc.sync.dma_start(h0T_sb, h0T[:].rearrange("c p n -> p c n"))
            h1T_sb = big.tile([128, 4, B], dt)
            nc.sync.dma_start(h1T_sb, h1T[:].rearrange("c p n -> p c n"))
            wih0_sb = big.tile([128, 12, 3 * HS], dt)
            nc.sync.dma_start(wih0_sb, wih0[:].rearrange("c p n -> p c n"))
            whh0_sb = big.tile([128, 4, 3 * HS], dt)
            nc.sync.dma_start(whh0_sb, whh0[:].rearrange("c p n -> p c n"))
            wih1_sb = big.tile([128, 4, 3 * HS], dt)
            nc.sync.dma_start(wih1_sb, wih1[:].rearrange("c p n -> p c n"))
            whh1_sb = big.tile([128, 4, 3 * HS], dt)
            nc.sync.dma_start(whh1_sb, whh1[:].rearrange("c p n -> p c n"))
            gbias_sb = big.tile([1, 4 * 3 * HS], dt)
            nc.sync.dma_start(gbias_sb, gbias[:])
            h0prev_sb = work.tile([B, HS], f32, name="h0prev_sb")
            nc.sync.dma_start(h0prev_sb, h0prev[:])
            h1prev_sb = work.tile([B, HS], f32, name="h1prev_sb")
            nc.sync.dma_start(h1prev_sb, h1prev[:])
            fcb_sb = big.tile([1, VS], dt)
            nc.sync.dma_start(fcb_sb, fcb[:])

            # ---- GRU gate matmuls that do not depend on attention ----
            # gh for both layers (from old hidden state) and the embedded part
            # of gi0. These overlap with the attention phase on the PE.
            gh0 = psum.tile([128, 512], f32, tag="pb", name="gh0")
            for j in range(4):
                nc.tensor.matmul(gh0[0:B, 0:3 * HS], lhsT=h0T_sb[:, j, :],
                                 rhs=whh0_sb[:, j, :], start=(j == 0), stop=False)
            nc.tensor.matmul(gh0[0:B, 0:3 * HS], lhsT=ones[0:1, 0:B],
                             rhs=gbias_sb[0:1, ds(192, 192)], start=False, stop=True)
            gh0_sb = work.tile([B, 3 * HS], f32, name="gh0_sb")
            nc.scalar.copy(gh0_sb, gh0[0:B, 0:3 * HS])

            gh1 = psum.tile([128, 512], f32, tag="pb", name="gh1")
            for j in range(4):
                nc.tensor.matmul(gh1[0:B, 0:3 * HS], lhsT=h1T_sb[:, j, :],
                                 rhs=whh1_sb[:, j, :], start=(j == 0), stop=False)
            nc.tensor.matmul(gh1[0:B, 0:3 * HS], lhsT=ones[0:1, 0:B],
                             rhs=gbias_sb[0:1, ds(3 * 192, 192)], start=False, stop=True)
            gh1_sb = work.tile([B, 3 * HS], f32, name="gh1_sb")
            nc.scalar.copy(gh1_sb, gh1[0:B, 0:3 * HS])

            # gi0: starts with embedded chunks; context chunks added after AG#1
            gi0 = psum.tile([128, 512], f32, tag="pb", name="gi0")
            for j in range(4):
                nc.tensor.matmul(gi0[0:B, 0:3 * HS], lhsT=embT_sb[:, j, :],
                                 rhs=wih0_sb[:, j, :], start=(j == 0), stop=False)
            nc.tensor.matmul(gi0[0:B, 0:3 * HS], lhsT=ones[0:1, 0:B],
                             rhs=gbias_sb[0:1, ds(0, 192)], start=False, stop=False)

            # ---- dbT[h, b] = (dec @ Wd + be + bd).T ----
            db_ps = psum.tile([128, 512], f32, tag="pb")
            dbv = db_ps[:, 0:4 * BL].rearrange("p (c b) -> p c b", b=BL)
            for i in range(4):
                for j in range(4):
                    nc.tensor.matmul(
                        dbv[:, i, :], lhsT=Wd_sb[:, j, ds(128 * i, 128)],
                        rhs=decT_sb[:, j, :], start=(j == 0), stop=False)
                nc.tensor.matmul(
                    dbv[:, i, :], lhsT=biasED_sb[0:1, ds(128 * i, 128)],
                    rhs=ones[0:1, 0:BL], start=False, stop=True)
            dbT_sb = big.tile([128, 4, BL], f32)
            nc.vector.tensor_copy(dbT_sb, dbv)

            # ---- energy_T = tanh(We.T @ enc + dbT) : [h, b*s] ----
            tanhE_sb = big.tile([128, 4, BL * S], dt)
            for c in range(2):
                for i in range(4):
                    e_ps = psum.tile([128, 512], f32, tag="pb", name="e_ps")
                    for j in range(8):
                        nc.tensor.matmul(
                            e_ps, lhsT=We_sb[:, j, ds(128 * i, 128)],
                            rhs=encT_sb[:, j, ds(512 * c, 512)],
                            start=(j == 0), stop=(j == 7))
                    ev = e_ps[:].rearrange("p (b s) -> p b s", s=S)
                    nc.vector.tensor_tensor(
                        ev, ev,
                        dbT_sb[:, i, ds(4 * c, 4), None].to_broadcast([128, 4, S]),
                        mybir.AluOpType.add)
                    nc.scalar.activation(
                        tanhE_sb[:, i, ds(512 * c, 512)], e_ps, Th)

            # ---- scores = v.T @ tanh_energy + mask bias; softmax ----
            scores_sb = work.tile([1, BL * S], f32)
            for c in range(2):
                sc_ps = psum.tile([128, 512], f32, tag="pb", name="sc_ps")
                for i in range(4):
                    nc.tensor.matmul(
                        sc_ps[0:1, :], lhsT=v_sb[:, i, :],
                        rhs=tanhE_sb[:, i, ds(512 * c, 512)],
                        start=(i == 0), stop=(i == 3))
                nc.vector.tensor_add(scores_sb[0:1, ds(512 * c, 512)],
                                     sc_ps[0:1, :], negm_sb[0:1, ds(512 * c, 512)])

            # reshape [1, b*s] -> [b, s] via sbuf-to-sbuf DMA
            scores_bs = work.tile([BL, S], f32)
            nc.gpsimd.dma_start(scores_bs, scores_sb)

            mx = work.tile([BL, 1], f32)
            nc.vector.reduce_max(mx, scores_bs, axis=mybir.AxisListType.X)
            shifted = work.tile([BL, S], f32)
            nc.vector.tensor_scalar(
                shifted, scores_bs, mx, None, mybir.AluOpType.subtract)
            exps = work.tile([BL, S], f32)
            nc.scalar.activation(exps, shifted, mybir.ActivationFunctionType.Exp)
            sm = work.tile([BL, 1], f32)
            nc.vector.reduce_sum(sm, exps, axis=mybir.AxisListType.X)
            rs = work.tile([BL, 1], f32)
            nc.vector.reciprocal(rs, sm)
            attn_sb = work.tile([BL, S], f32)
            nc.vector.tensor_scalar_mul(attn_sb, exps, rs)
            nc.gpsimd.dma_start(attn_o[:], attn_sb)

            # attn.T [s, b] for context matmul
            at_ps = psum.tile([128, 512], f32, tag="pb", name="at_ps")
            nc.tensor.transpose(at_ps[:, 0:BL], attn_sb, ident[0:BL, 0:BL])
            attnT_sb = work.tile([128, BL], dt)
            nc.vector.tensor_copy(attnT_sb, at_ps[:, 0:BL])

            # ---- context[b, e] = attn[b, :] @ enc[b]; M=1 matmuls, 1 DMA out ----
            ctx_in = dram.tile([BL, ENC], f32)
            ctx_out = dram.tile([B, ENC], f32, addr_space="Shared")
            ctx_cat = work.tile([1, BL * ENC], f32)
            for b in range(BL):
                encb = encn_pool.tile([128, ENC], dt, tag="encn")
                nc.sync.dma_start(encb, encN[:, b, :])
                for c2 in range(2):
                    c_ps = psum.tile([128, 512], f32, tag="pb", name="c_ps")
                    nc.tensor.matmul(
                        c_ps[0:1, :], lhsT=attnT_sb[:, b:b + 1],
                        rhs=encb[:, ds(512 * c2, 512)], start=True, stop=True)
                    nc.vector.tensor_copy(
                        ctx_cat[0:1, ds(1024 * b + 512 * c2, 512)], c_ps[0:1, :])
            nc.gpsimd.dma_start(ctx_in[:], ctx_cat)
            AG(ctx_in, ctx_out)
            ctx_full = work.tile([B, ENC], f32)
            nc.gpsimd.dma_start(ctx_full, ctx_out[:])
            ctxT_sb = big.tile([128, 8, B], dt)
            for t in range(8):
                t_ps = psum.tile([128, 512], f32, tag="pb", name="t_ps")
                nc.tensor.transpose(
                    t_ps[:, 0:B], ctx_full[:, ds(128 * t, 128)], ident[0:B, 0:B])
                nc.vector.tensor_copy(ctxT_sb[:, t, :], t_ps[:, 0:B])

            def gru_gates(gi, gh_sb, hprev_sb, lname):
                pre_r = work.tile([B, HS], f32, name=f"pre_r_{lname}")
                nc.vector.tensor_add(pre_r, gi[0:B, 0:HS], gh_sb[:, 0:HS])
                r = work.tile([B, HS], f32, name=f"r_{lname}")
                nc.scalar.activation(r, pre_r, Sg)
                pre_z = work.tile([B, HS], f32, name=f"pre_z_{lname}")
                nc.vector.tensor_add(pre_z, gi[0:B, HS:2 * HS], gh_sb[:, HS:2 * HS])
                z = work.tile([B, HS], f32, name=f"z_{lname}")
                nc.scalar.activation(z, pre_z, Sg)
                pre_n = work.tile([B, HS], f32, name=f"pre_n_{lname}")
                nc.vector.tensor_mul(pre_n, r, gh_sb[:, 2 * HS:3 * HS])
                nc.vector.tensor_add(pre_n, pre_n, gi[0:B, 2 * HS:3 * HS])
                n = work.tile([B, HS], f32, name=f"n_{lname}")
                nc.scalar.activation(n, pre_n, Th)
                hnew = work.tile([B, HS], f32, name=f"hnew_{lname}")
                nc.vector.tensor_sub(hnew, hprev_sb, n)
                nc.vector.tensor_mul(hnew, hnew, z)
                nc.vector.tensor_add(hnew, hnew, n)
                return hnew

            def transpose_ag(h_sb, dram_in, dram_out, out_sb, lname):
                # [B, HS] f32 -> transpose -> cast dt -> AllGather -> [128, 4, B] dt
                tp = psum.tile([128, 512], f32, tag="pb", name=f"tp_{lname}")
                nc.tensor.transpose(tp[0:HS, 0:B], h_sb, ident[0:B, 0:B])
                hT_l = work.tile([HS, B], dt, name=f"hT_{lname}")
                nc.vector.tensor_copy(hT_l, tp[0:HS, 0:B])
                nc.gpsimd.dma_start(dram_in[:], hT_l)
                AG(dram_in, dram_out)
                nc.gpsimd.dma_start(out_sb, dram_out[:].rearrange("(c p) b -> p c b", p=128))

            # ---- GRU layer 0: add context chunks to gi0, then gates ----
            for t in range(8):
                nc.tensor.matmul(gi0[0:B, 0:3 * HS], lhsT=ctxT_sb[:, t, :],
                                 rhs=wih0_sb[:, 4 + t, :], start=False,
                                 stop=(t == 7))
            h0n = gru_gates(gi0, gh0_sb, h0prev_sb, "l0")
            nc.gpsimd.dma_start(h0_o[:], h0n)
            h0T_in = dram.tile([HS, B], dt)
            h0T_out = dram.tile([H, B], dt, addr_space="Shared")
            h0nT_sb = big.tile([128, 4, B], dt)
            transpose_ag(h0n, h0T_in, h0T_out, h0nT_sb, "l0")

            # ---- GRU layer 1 ----
            gi1 = psum.tile([128, 512], f32, tag="pb", name="gi1")
            for j in range(4):
                nc.tensor.matmul(gi1[0:B, 0:3 * HS], lhsT=h0nT_sb[:, j, :],
                                 rhs=wih1_sb[:, j, :], start=(j == 0), stop=False)
            nc.tensor.matmul(gi1[0:B, 0:3 * HS], lhsT=ones[0:1, 0:B],
                             rhs=gbias_sb[0:1, ds(2 * 192, 192)], start=False, stop=True)
            h1n = gru_gates(gi1, gh1_sb, h1prev_sb, "l1")
            nc.gpsimd.dma_start(h1_o[:], h1n)
            h1T_in = dram.tile([HS, B], dt)
            h1T_out = dram.tile([H, B], dt, addr_space="Shared")
            h1nT_sb = big.tile([128, 4, B], dt)
            transpose_ag(h1n, h1T_in, h1T_out, h1nT_sb, "l1")

            # ---- fc_out: logits[b, v] = out_in @ fc_w_slice.T + fc_b ----
            # out_in = [h1 | context | embedded]; chunk order matches host fcwT
            chunks = ([h1nT_sb[:, j, :] for j in range(4)]
                      + [ctxT_sb[:, t, :] for t in range(8)]
                      + [embT_sb[:, j, :] for j in range(4)])
            fc_ps = [psum.tile([128, 512], f32, tag="pb", name=f"fc_ps{v}")
                     for v in range(8)]
            for kc in range(16):
                w_t = fcw_pool.tile([128, VS], dt, tag="fcw")
                nc.sync.dma_start(w_t, fcwT[kc])
                for v in range(8):
                    nc.tensor.matmul(
                        fc_ps[v][0:B, 0:VT], lhsT=chunks[kc],
                        rhs=w_t[:, ds(VT * v, VT)], start=(kc == 0), stop=False)
            for v in range(8):
                nc.tensor.matmul(
                    fc_ps[v][0:B, 0:VT], lhsT=ones[0:1, 0:B],
                    rhs=fcb_sb[0:1, ds(VT * v, VT)], start=False, stop=True)
                lg = drainp.tile([B, VT], f32, tag="lg")
                nc.vector.tensor_copy(lg, fc_ps[v][0:B, 0:VT])
                nc.sync.dma_start(logits_o[:, ds(VT * v, VT)], lg)

    nc.finalize()
    return nc


_PROGRAM_CACHE = {}


def _get_program():
    n_fcw_bufs = 2 if DT == F32 else 10
    key = (str(DT), n_fcw_bufs)
    if key not in _PROGRAM_CACHE:
        _PROGRAM_CACHE[key] = _build_program(DT, n_fcw_bufs)
    return _PROGRAM_CACHE[key]


def _prep_core_inputs(i, input_ids, hidden, enc, mask, emb, att_We, att_be,
                      att_Wd, att_bd, att_v, w_ih0, w_hh0, b_ih0, b_hh0,
                      w_ih1, w_hh1, b_ih1, b_hh1, fc_w, fc_b, embedded):
    f4 = np.float32
    bsl = slice(BL * i, BL * (i + 1))
    vsl = slice(VS * i, VS * (i + 1))
    hsl = np.arange(HS * i, HS * (i + 1))
    rows = np.concatenate([hsl, H + hsl, 2 * H + hsl])

    enc_sl = enc[bsl]                                    # [8, S, ENC]
    c = np.ascontiguousarray

    def d(x):
        return c(x.astype(DT_NP))

    m = {}
    m["encT"] = d(enc_sl.transpose(2, 0, 1).reshape(8, 128, BL * S))
    m["encN"] = d(enc_sl.transpose(1, 0, 2))
    m["We"] = d(att_We.reshape(8, 128, H))
    wd = att_Wd.reshape(4, 128, H)
    dct = hidden[1][bsl].T.reshape(4, 128, BL)
    av = att_v.reshape(4, 128, 1)
    m["attp"] = d(np.concatenate([wd, dct, av], axis=2))
    m["biasED"] = d((att_be + att_bd).reshape(1, H))
    m["negm"] = c(np.where(mask[bsl] > 0, 0.0, -1e9).astype(f4).reshape(1, BL * S))
    m["embT"] = d(embedded.T.reshape(4, 128, B))
    m["h0T"] = d(hidden[0].T.reshape(4, 128, B))
    m["h1T"] = d(hidden[1].T.reshape(4, 128, B))
    m["h0prev"] = c(hidden[0][:, hsl].astype(f4))
    m["h1prev"] = c(hidden[1][:, hsl].astype(f4))
    m["wih0"] = d(w_ih0[rows].T.reshape(12, 128, 3 * HS))
    m["whh0"] = d(w_hh0[rows].T.reshape(4, 128, 3 * HS))
    m["wih1"] = d(w_ih1[rows].T.reshape(4, 128, 3 * HS))
    m["whh1"] = d(w_hh1[rows].T.reshape(4, 128, 3 * HS))
    m["gbias"] = d(np.concatenate(
        [b_ih0[rows], b_hh0[rows], b_ih1[rows], b_hh1[rows]]).reshape(1, 4 * 3 * HS))
    # fc_w columns: [h1 | context | embedded] == natural out_in order
    m["fcwT"] = d(fc_w[vsl].T.reshape(16, 128, VS))
    m["fcb"] = d(fc_b[vsl].reshape(1, VS))
    return m


def kernel(input_ids, hidden, encoder_outputs, mask, emb, att_We, att_be,
           att_Wd, att_bd, att_v, w_ih0, w_hh0, b_ih0, b_hh0,
           w_ih1, w_hh1, b_ih1, b_hh1, fc_w, fc_b, _trace=False):
    f4 = np.float32
    input_ids = np.asarray(input_ids)
    hidden = np.asarray(hidden, dtype=f4)
    enc = np.asarray(encoder_outputs, dtype=f4)
    mask = np.asarray(mask)
    emb = np.asarray(emb, dtype=f4)
    att_We = np.asarray(att_We, dtype=f4)
    att_be = np.asarray(att_be, dtype=f4)
    att_Wd = np.asarray(att_Wd, dtype=f4)
    att_bd = np.asarray(att_bd, dtype=f4)
    att_v = np.asarray(att_v, dtype=f4)
    w_ih0 = np.asarray(w_ih0, dtype=f4)
    w_hh0 = np.asarray(w_hh0, dtype=f4)
    b_ih0 = np.asarray(b_ih0, dtype=f4)
    b_hh0 = np.asarray(b_hh0, dtype=f4)
    w_ih1 = np.asarray(w_ih1, dtype=f4)
    w_hh1 = np.asarray(w_hh1, dtype=f4)
    b_ih1 = np.asarray(b_ih1, dtype=f4)
    b_hh1 = np.asarray(b_hh1, dtype=f4)
    fc_w = np.asarray(fc_w, dtype=f4)
    fc_b = np.asarray(fc_b, dtype=f4)

    embedded = emb[input_ids.astype(np.int64)]           # [B, E] host gather

    nc = _get_program()
    in_maps = [
        _prep_core_inputs(i, input_ids, hidden, enc, mask, emb, att_We, att_be,
                          att_Wd, att_bd, att_v, w_ih0, w_hh0, b_ih0, b_hh0,
                          w_ih1, w_hh1, b_ih1, b_hh1, fc_w, fc_b, embedded)
        for i in range(NCORES)
    ]
    res = run_bass_kernel_spmd(nc, in_maps, core_ids=list(range(NCORES)),
                               trace=_trace)
    outs = res.results

    logits = np.concatenate([outs[i]["logits"] for i in range(NCORES)], axis=1)
    h0 = np.concatenate([outs[i]["h0o"] for i in range(NCORES)], axis=1)
    h1 = np.concatenate([outs[i]["h1o"] for i in range(NCORES)], axis=1)
    new_hidden = np.stack([h0, h1], axis=0)
    attn = np.concatenate([outs[i]["attno"] for i in range(NCORES)], axis=0)

    kernel._last_exec_time_ns = res.exec_time_ns
    kernel._last_res = res
    return logits.astype(f4), new_hidden.astype(f4), attn.astype(f4)
